# revision 13
# baseline (speedup 1.0000x reference)
"""KAN encoder (2 KAN layers + relu + linear head) on 8 trn2 NeuronCores.

Strategy: data-parallel on batch (512 rows/core), all weights replicated,
feature-on-partition / batch-on-free layout throughout (no transposes).

The spline path is a dense matmul over (in*9) with B-spline bases computed
via the exact identity

    bases_k(x) * 6 = a^3 - 4*e^3,   a = relu(2 - s_k),  e = relu(a - 1),
    s_k = |2.5*x + 3.5 - k|

(the 1/6 and the spline_scaler are folded into the weights host-side; the
*4 is realized as (2e)^2 * e).

Performance notes vs the f32 baseline (720us):
  * weights, x and all feature tensors are fp16: halves HBM traffic and
    matmuls still run at 1 cycle/row (same as f32r).
  * the basis slices share relu scale/bias, so the relu/cube chain runs as
    WIDE [128, 4*512] ops: ACT does the 8 per-k Abs + silu + packed relus,
    DVE runs packed fp16 tensor_scalar (4x_2p) / tensor_tensor (2x_1p) ops.
    The f32 baseline was vector-bound (DVE 93% busy at 533ns per
    tensor_tensor); here DVE sits at ~60% and the PE at ~93%.
  * layer-0 features are computed once and kept in SBUF for both output
    halves (the baseline recomputed them).
  * layer-0 og0 runs in two phases: all j=0 (silu-path) matmuls for the 8
    chunks first - they only need silu(x) and a small weight block - so
    the PE starts within ~4us while the spline chains fill the pipeline.
  * features live in 3 tiles per chunk (silu slot + two 4-slot spline
    halves) and matmuls run j-major, so matmuls start as soon as each
    piece is ready instead of waiting for the full feature tile.
"""
import numpy as np
from contextlib import ExitStack

from concourse import bacc, tile, mybir
from concourse.bass_utils import run_bass_kernel_spmd

F32 = mybir.dt.float32
F16 = mybir.dt.float16
AF = mybir.ActivationFunctionType
ALU = mybir.AluOpType

B, D_IN, H0, H1, L = 4096, 1024, 2048, 1024, 512
NCORES = 8
BC = B // NCORES          # 512 batch cols per core
NB = 512                  # free-dim (batch) tile = full per-core batch
HW = 4 * NB               # half of the 8 spline slots

_cache = {}


def _build_full():
    nc = bacc.Bacc("TRN2", target_bir_lowering=False, debug=False,
                   num_devices=NCORES)

    x_d = nc.dram_tensor("x_d", [8, 128, BC], F16, kind="ExternalInput")
    w0_d = nc.dram_tensor("w0_d", [8, 2, 2, 128, 9 * NB], F16,
                          kind="ExternalInput")
    w1_d = nc.dram_tensor("w1_d", [16, 2, 128, 9 * NB], F16,
                          kind="ExternalInput")
    dw_d = nc.dram_tensor("dw_d", [8, 128, L], F16, kind="ExternalInput")
    db_d = nc.dram_tensor("db_d", [128, 4], F32, kind="ExternalInput")
    o_d = nc.dram_tensor("o_d", [4, 128, BC], F32, kind="ExternalOutput")

    with tile.TileContext(nc) as tc, ExitStack() as ctx:
        psum = ctx.enter_context(tc.tile_pool(name="psum", bufs=1,
                                              space="PSUM"))
        fpool = ctx.enter_context(tc.tile_pool(name="fpool", bufs=1))
        wpool = ctx.enter_context(tc.tile_pool(name="wpool", bufs=1))
        ipool = ctx.enter_context(tc.tile_pool(name="ipool", bufs=1))
        opool = ctx.enter_context(tc.tile_pool(name="opool", bufs=1))

        fics = [None] * 8     # current feats tiles (base, sp0, sp1)
        h0ts = [None] * 16    # layer-0 output chunks

        bias_tiles = {}

        def bias_ap(val):
            val = float(val)
            if val not in bias_tiles:
                t = opool.tile([128, 1], F32, tag=f"b{len(bias_tiles)}",
                               name=f"bias{len(bias_tiles)}")
                nc.gpsimd.memset(t[:, :], val)
                bias_tiles[val] = t
            return bias_tiles[val][:, :]

        def emit_base(src_ap, tagp):
            """base slot (128, NB) = silu(src)."""
            fb = fpool.tile([128, NB], F16, tag=f"fb{tagp[-1]}",
                            name=f"fb{tagp}")
            nc.scalar.activation(fb[:, :], src_ap, AF.Silu,
                                 bias=bias_ap(0.0), scale=1.0)
            return fb

        def emit_spline(src_ap, tagp):
            """Returns [sp0, sp1]: spP (128,4*NB) slot q -> 6*bases_{4P+q}."""
            out = []
            for p in range(2):
                # sp shares tag q{p} with a3: sp is dead once av is made
                sp = ipool.tile([128, HW], F16, tag=f"q{p}", bufs=1,
                                name=f"sp{p}_{tagp}")
                for q in range(4):
                    k = 4 * p + q
                    nc.scalar.activation(sp[:, q * NB:(q + 1) * NB], src_ap,
                                         AF.Abs, bias=bias_ap(3.5 - k),
                                         scale=2.5)
                av = ipool.tile([128, HW], F16, tag=f"av{p}", bufs=1,
                                name=f"av{p}_{tagp}")
                nc.scalar.activation(av[:, :], sp[:, :], AF.Relu,
                                     bias=bias_ap(2.0), scale=-1.0)
                et = ipool.tile([128, HW], F16, tag=f"et{p}", bufs=1,
                                name=f"et{p}_{tagp}")
                nc.vector.tensor_scalar(et[:, :], av[:, :], 1.0, 0.0,
                                        ALU.subtract, ALU.max)
                a2 = ipool.tile([128, HW], F16, tag=f"p{p}", bufs=1,
                                name=f"a2{p}_{tagp}")
                nc.vector.tensor_tensor(a2[:, :], av[:, :], av[:, :],
                                        ALU.mult)
                a3 = ipool.tile([128, HW], F16, tag=f"q{p}", bufs=1,
                                name=f"a3{p}_{tagp}")
                nc.vector.tensor_tensor(a3[:, :], a2[:, :], av[:, :],
                                        ALU.mult)
                gt = ipool.tile([128, HW], F16, tag=f"p{p}", bufs=1,
                                name=f"gt{p}_{tagp}")
                nc.vector.tensor_scalar(gt[:, :], et[:, :], 2.0, None,
                                        ALU.mult)
                e2 = ipool.tile([128, HW], F16, tag=f"r{p}", bufs=1,
                                name=f"e2{p}_{tagp}")
                nc.vector.tensor_tensor(e2[:, :], gt[:, :], gt[:, :],
                                        ALU.mult)
                e3 = ipool.tile([128, HW], F16, tag=f"p{p}", bufs=1,
                                name=f"e3{p}_{tagp}")
                nc.vector.tensor_tensor(e3[:, :], e2[:, :], et[:, :],
                                        ALU.mult)
                fs = fpool.tile([128, HW], F16, tag=f"fs{p}_{tagp[-1]}",
                                name=f"fs{p}_{tagp}")
                nc.vector.tensor_tensor(fs[:, :], a3[:, :], e3[:, :],
                                        ALU.subtract)
                out.append(fs)
            return out

        def kan_matmuls(pts, whs, fic, ic, n_ic, js=range(9), j_off=0,
                        has_start=True):
            # j-major so early-ready features feed the PE first
            for j in js:
                if j == 0:
                    rhs = fic[0][:, :]
                else:
                    rhs = fic[1 + (j - 1) // 4][
                        :, ((j - 1) % 4) * NB:((j - 1) % 4 + 1) * NB]
                for half in range(2):
                    ws = whs[half]
                    for oc4 in range(4):
                        oc = half * 4 + oc4
                        base = (j - j_off) * NB + oc4 * 128
                        nc.tensor.matmul(
                            pts[oc][:, :], ws[:, base:base + 128], rhs,
                            start=(has_start and ic == 0 and j == 0),
                            stop=(ic == n_ic - 1 and j == 8))

        # ---- Layer 0: out split in two groups of 8 PSUM banks ----
        for og in range(2):
            pts = [psum.tile([128, NB], F32, tag=f"bank{oc}",
                             name=f"psA{og}_{oc}") for oc in range(8)]
            if og == 0:
                # phase A: j=0 (silu-path) matmuls only; tiny weight DMAs
                xts = []
                for ic in range(8):
                    xt = ipool.tile([128, NB], F16, tag=f"xt{ic}",
                                    name=f"xt{ic}")
                    nc.sync.dma_start(out=xt[:, :], in_=x_d[ic, :, :])
                    xts.append(xt)
                    fics[ic] = [emit_base(xt[:, :], f"a{ic}"), None, None]
                    wbs = []
                    for half in range(2):
                        wb = wpool.tile([128, NB], F16,
                                        tag=f"wb{ic}_{half}",
                                        name=f"wb{ic}_{half}")
                        nc.sync.dma_start(out=wb[:, :],
                                          in_=w0_d[ic, 0, half, :, 0:NB])
                        wbs.append(wb)
                    kan_matmuls(pts, wbs, fics[ic], ic, 8, js=(0,))
                # phase B: spline chains + j=1..8 matmuls, streamed weights
                for ic in range(8):
                    fics[ic][1:] = emit_spline(xts[ic][:, :], f"a{ic}")
                    wsr = []
                    for half in range(2):
                        wt = wpool.tile([128, 8 * NB], F16, tag="wsr",
                                        bufs=2, name=f"wsr{ic}_{half}")
                        nc.sync.dma_start(out=wt[:, :],
                                          in_=w0_d[ic, 0, half, :, NB:9 * NB])
                        wsr.append(wt)
                    kan_matmuls(pts, wsr, fics[ic], ic, 8,
                                js=range(1, 9), j_off=1)
            else:
                for ic in range(8):
                    whs = []
                    for half in range(2):
                        wt = wpool.tile([128, 9 * NB], F16, tag="ws",
                                        bufs=2, name=f"w1g_{ic}_{half}")
                        nc.sync.dma_start(out=wt[:, :],
                                          in_=w0_d[ic, 1, half, :, :])
                        whs.append(wt)
                    kan_matmuls(pts, whs, fics[ic], ic, 8)
                    # recompute feats for layer 1 while og1 matmuls drain
                    fics[ic] = [emit_base(h0ts[ic][:, :], f"b{ic}")] + \
                        emit_spline(h0ts[ic][:, :], f"b{ic}")
            for oc in range(8):
                h0t = fpool.tile([128, NB], F16, tag=f"h0_{og * 8 + oc}",
                                 name=f"h0t{og * 8 + oc}")
                nc.scalar.activation(h0t[:, :], pts[oc][:, :], AF.Copy,
                                     bias=0.0, scale=1.0)
                h0ts[og * 8 + oc] = h0t

        # head weights: small, load while layer 1 runs
        dwt = opool.tile([128, 8, L], F16, name="dwt")
        for ic in range(8):
            nc.sync.dma_start(out=dwt[:, ic, :], in_=dw_d[ic, :, :])
        dbt = opool.tile([128, 4], F32, name="dbt")
        nc.sync.dma_start(out=dbt[:, :], in_=db_d[:, :])

        # ---- Layer 1: 8 out chunks, 16 contraction chunks ----
        pts = [psum.tile([128, NB], F32, tag=f"bank{oc}", name=f"psB{oc}")
               for oc in range(8)]
        for ic in range(16):
            if ic >= 8:
                fics[ic - 8] = [emit_base(h0ts[ic][:, :], f"c{ic - 8}")] + \
                    emit_spline(h0ts[ic][:, :], f"c{ic - 8}")
            whs = []
            for half in range(2):
                wt = wpool.tile([128, 9 * NB], F16, tag="ws", bufs=2,
                                name=f"w1_{ic}_{half}")
                nc.sync.dma_start(out=wt[:, :], in_=w1_d[ic, half, :, :])
                whs.append(wt)
            kan_matmuls(pts, whs, fics[ic % 8], ic, 16)

        # ---- Head: relu(h1) @ dw.T + db ----
        rls = []
        for oc in range(8):
            rl = opool.tile([128, NB], F16, tag=f"rl{oc}", name=f"rl{oc}")
            if oc % 2 == 0:
                nc.scalar.activation(rl[:, :], pts[oc][:, :], AF.Relu,
                                     bias=bias_ap(0.0), scale=1.0)
            else:
                nc.vector.tensor_scalar(rl[:, :], pts[oc][:, :], 0.0, None,
                                        ALU.max)
            rls.append(rl)
        hpts = [psum.tile([128, NB], F32, tag=f"bank{lc}", name=f"psC{lc}")
                for lc in range(4)]
        for ic in range(8):
            for lc in range(4):
                nc.tensor.matmul(hpts[lc][:, :],
                                 dwt[:, ic, lc * 128:(lc + 1) * 128],
                                 rls[ic][:, :],
                                 start=(ic == 0), stop=(ic == 7))
        for lc in range(4):
            ot = opool.tile([128, NB], F32, tag="ot", bufs=2, name=f"ot{lc}")
            nc.scalar.activation(ot[:, :], hpts[lc][:, :], AF.Identity,
                                 bias=dbt[:, lc:lc + 1], scale=1.0)
            nc.sync.dma_start(out=o_d[lc, :, :], in_=ot[:, :])

    nc.compile()
    return nc


def _prep_weights(bw0, sw0, ss0, bw1, sw1, ss1, dw, db):
    # KAN layer weights: slot j=0 -> base weight, j=1+k -> sw*ss/6
    W0 = np.empty((D_IN, 9, H0), np.float32)
    W0[:, 0, :] = bw0.T
    W0[:, 1:, :] = (sw0 * (ss0[:, :, None] / 6.0)).transpose(1, 2, 0)
    w0 = np.ascontiguousarray(
        W0.reshape(8, 128, 9, 2, 2, 4, 128)
          .transpose(0, 3, 4, 1, 2, 5, 6)
          .reshape(8, 2, 2, 128, 9 * NB)).astype(np.float16)
    W1 = np.empty((H0, 9, H1), np.float32)
    W1[:, 0, :] = bw1.T
    W1[:, 1:, :] = (sw1 * (ss1[:, :, None] / 6.0)).transpose(1, 2, 0)
    w1 = np.ascontiguousarray(
        W1.reshape(16, 128, 9, 2, 4, 128)
          .transpose(0, 3, 1, 2, 4, 5)
          .reshape(16, 2, 128, 9 * NB)).astype(np.float16)
    dwt = np.ascontiguousarray(dw.T.reshape(8, 128, L)).astype(np.float16)
    dbt = np.ascontiguousarray(db.reshape(4, 128).T.astype(np.float32))
    return w0, w1, dwt, dbt


def kernel(x, bw0, sw0, ss0, bw1, sw1, ss1, dw, db):
    if "nc" not in _cache:
        _cache["nc"] = _build_full()
    nc = _cache["nc"]
    w0, w1, dwt, dbt = _prep_weights(
        np.asarray(bw0, np.float32), np.asarray(sw0, np.float32),
        np.asarray(ss0, np.float32), np.asarray(bw1, np.float32),
        np.asarray(sw1, np.float32), np.asarray(ss1, np.float32),
        np.asarray(dw, np.float32), np.asarray(db, np.float32))
    xT = np.ascontiguousarray(np.asarray(x, np.float32).T).astype(np.float16)
    in_maps = []
    for c in range(NCORES):
        xc = np.ascontiguousarray(
            xT[:, c * BC:(c + 1) * BC].reshape(8, 128, BC))
        in_maps.append({"x_d": xc, "w0_d": w0, "w1_d": w1,
                        "dw_d": dwt, "db_d": dbt})
    _cache["in_maps"] = in_maps
    res = run_bass_kernel_spmd(nc, in_maps, list(range(NCORES)))
    out = np.empty((B, L), np.float32)
    for c in range(NCORES):
        oc = res.results[c]["o_d"]          # (4, 128, BC)
        out[c * BC:(c + 1) * BC, :] = oc.reshape(L, BC).T
    return out


# revision 16
# speedup vs baseline: 1.0300x; 1.0300x over previous
"""KAN encoder (2 KAN layers + relu + linear head) on 8 trn2 NeuronCores.

Strategy: data-parallel on batch (512 rows/core), all weights replicated,
feature-on-partition / batch-on-free layout throughout (no transposes).

The spline path is a dense matmul over (in*9) with B-spline bases computed
via the exact identity

    bases_k(x) * 6 = a^3 - 4*e^3,   a = relu(2 - s_k),  e = relu(a - 1),
    s_k = |2.5*x + 3.5 - k|

(the 1/6 and the spline_scaler are folded into the weights host-side; the
*4 is realized as (2e)^2 * e).

Performance notes vs the f32 baseline (720us):
  * weights, x and all feature tensors are fp16: halves HBM traffic and
    matmuls still run at 1 cycle/row (same as f32r).
  * the basis slices share relu scale/bias, so the relu/cube chain runs as
    WIDE [128, 4*512] ops: ACT does the 8 per-k Abs + silu + packed relus,
    DVE runs packed fp16 tensor_scalar (4x_2p) / tensor_tensor (2x_1p) ops.
    The f32 baseline was vector-bound (DVE 93% busy at 533ns per
    tensor_tensor); here DVE sits at ~60% and the PE at ~93%.
  * layer-0 features are computed once and kept in SBUF for both output
    halves (the baseline recomputed them).
  * layer-0 og0 runs in two phases: all j=0 (silu-path) matmuls for the 8
    chunks first - they only need silu(x) and a small weight block - so
    the PE starts within ~4us while the spline chains fill the pipeline.
  * features live in 3 tiles per chunk (silu slot + two 4-slot spline
    halves) and matmuls run j-major, so matmuls start as soon as each
    piece is ready instead of waiting for the full feature tile.
"""
import numpy as np
from contextlib import ExitStack

from concourse import bacc, tile, mybir
from concourse.bass_utils import run_bass_kernel_spmd

F32 = mybir.dt.float32
F16 = mybir.dt.float16
AF = mybir.ActivationFunctionType
ALU = mybir.AluOpType

B, D_IN, H0, H1, L = 4096, 1024, 2048, 1024, 512
NCORES = 8
BC = B // NCORES          # 512 batch cols per core
NB = 512                  # free-dim (batch) tile = full per-core batch
HW = 4 * NB               # half of the 8 spline slots

_cache = {}


def _build_full():
    nc = bacc.Bacc("TRN2", target_bir_lowering=False, debug=False,
                   num_devices=NCORES)

    x_d = nc.dram_tensor("x_d", [8, 128, BC], F16, kind="ExternalInput")
    w0_d = nc.dram_tensor("w0_d", [8, 2, 2, 128, 9 * NB], F16,
                          kind="ExternalInput")
    w1_d = nc.dram_tensor("w1_d", [16, 2, 128, 9 * NB], F16,
                          kind="ExternalInput")
    dw_d = nc.dram_tensor("dw_d", [8, 128, L], F16, kind="ExternalInput")
    db_d = nc.dram_tensor("db_d", [128, 4], F32, kind="ExternalInput")
    o_d = nc.dram_tensor("o_d", [4, 128, BC], F32, kind="ExternalOutput")

    with tile.TileContext(nc) as tc, ExitStack() as ctx:
        psum = ctx.enter_context(tc.tile_pool(name="psum", bufs=1,
                                              space="PSUM"))
        fpool = ctx.enter_context(tc.tile_pool(name="fpool", bufs=1))
        wpool = ctx.enter_context(tc.tile_pool(name="wpool", bufs=1))
        ipool = ctx.enter_context(tc.tile_pool(name="ipool", bufs=1))
        opool = ctx.enter_context(tc.tile_pool(name="opool", bufs=1))

        fics = [None] * 8     # current feats tiles (base, sp0, sp1)
        h0ts = [None] * 16    # layer-0 output chunks

        bias_tiles = {}

        def bias_ap(val):
            val = float(val)
            if val not in bias_tiles:
                t = opool.tile([128, 1], F32, tag=f"b{len(bias_tiles)}",
                               name=f"bias{len(bias_tiles)}")
                nc.gpsimd.memset(t[:, :], val)
                bias_tiles[val] = t
            return bias_tiles[val][:, :]

        def emit_base(src_ap, tagp):
            """base slot (128, NB) = silu(src)."""
            fb = fpool.tile([128, NB], F16, tag=f"fb{tagp[-1]}",
                            name=f"fb{tagp}")
            nc.scalar.activation(fb[:, :], src_ap, AF.Silu,
                                 bias=bias_ap(0.0), scale=1.0)
            return fb

        def emit_spline(src_ap, tagp):
            """Returns [sp0, sp1]: spP (128,4*NB) slot q -> 6*bases_{4P+q}."""
            out = []
            for p in range(2):
                # sp is ACT-produced and ACT-consumed: bufs=1 cannot stall
                # the cross-engine pipeline; av is read by DVE so it gets 2.
                sp = ipool.tile([128, HW], F16, tag=f"sp{p}", bufs=1,
                                name=f"sp{p}_{tagp}")
                for q in range(4):
                    k = 4 * p + q
                    nc.scalar.activation(sp[:, q * NB:(q + 1) * NB], src_ap,
                                         AF.Abs, bias=bias_ap(3.5 - k),
                                         scale=2.5)
                av = ipool.tile([128, HW], F16, tag=f"av{p}", bufs=2,
                                name=f"av{p}_{tagp}")
                nc.scalar.activation(av[:, :], sp[:, :], AF.Relu,
                                     bias=bias_ap(2.0), scale=-1.0)
                et = ipool.tile([128, HW], F16, tag=f"et{p}", bufs=1,
                                name=f"et{p}_{tagp}")
                nc.vector.tensor_scalar(et[:, :], av[:, :], 1.0, 0.0,
                                        ALU.subtract, ALU.max)
                a2 = ipool.tile([128, HW], F16, tag=f"p{p}", bufs=1,
                                name=f"a2{p}_{tagp}")
                nc.vector.tensor_tensor(a2[:, :], av[:, :], av[:, :],
                                        ALU.mult)
                a3 = ipool.tile([128, HW], F16, tag=f"q{p}", bufs=1,
                                name=f"a3{p}_{tagp}")
                nc.vector.tensor_tensor(a3[:, :], a2[:, :], av[:, :],
                                        ALU.mult)
                gt = ipool.tile([128, HW], F16, tag=f"p{p}", bufs=1,
                                name=f"gt{p}_{tagp}")
                nc.vector.tensor_scalar(gt[:, :], et[:, :], 2.0, None,
                                        ALU.mult)
                e2 = ipool.tile([128, HW], F16, tag=f"r{p}", bufs=1,
                                name=f"e2{p}_{tagp}")
                nc.vector.tensor_tensor(e2[:, :], gt[:, :], gt[:, :],
                                        ALU.mult)
                e3 = ipool.tile([128, HW], F16, tag=f"p{p}", bufs=1,
                                name=f"e3{p}_{tagp}")
                nc.vector.tensor_tensor(e3[:, :], e2[:, :], et[:, :],
                                        ALU.mult)
                fs = fpool.tile([128, HW], F16, tag=f"fs{p}_{tagp[-1]}",
                                name=f"fs{p}_{tagp}")
                nc.vector.tensor_tensor(fs[:, :], a3[:, :], e3[:, :],
                                        ALU.subtract)
                out.append(fs)
            return out

        def kan_matmuls(pts, whs, fic, ic, n_ic, js=range(9), j_off=0,
                        has_start=True):
            # j-major so early-ready features feed the PE first
            for j in js:
                if j == 0:
                    rhs = fic[0][:, :]
                else:
                    rhs = fic[1 + (j - 1) // 4][
                        :, ((j - 1) % 4) * NB:((j - 1) % 4 + 1) * NB]
                for half in range(2):
                    ws = whs[half]
                    for oc4 in range(4):
                        oc = half * 4 + oc4
                        base = (j - j_off) * NB + oc4 * 128
                        nc.tensor.matmul(
                            pts[oc][:, :], ws[:, base:base + 128], rhs,
                            start=(has_start and ic == 0 and j == 0),
                            stop=(ic == n_ic - 1 and j == 8))

        # ---- Layer 0: out split in two groups of 8 PSUM banks ----
        for og in range(2):
            pts = [psum.tile([128, NB], F32, tag=f"bank{oc}",
                             name=f"psA{og}_{oc}") for oc in range(8)]
            if og == 0:
                # phase A: j=0 (silu-path) matmuls only; tiny weight DMAs
                xts = []
                for ic in range(8):
                    xt = ipool.tile([128, NB], F16, tag=f"xt{ic}",
                                    name=f"xt{ic}")
                    nc.sync.dma_start(out=xt[:, :], in_=x_d[ic, :, :])
                    xts.append(xt)
                    fics[ic] = [emit_base(xt[:, :], f"a{ic}"), None, None]
                    wbs = []
                    for half in range(2):
                        wb = wpool.tile([128, NB], F16,
                                        tag=f"wb{ic}_{half}",
                                        name=f"wb{ic}_{half}")
                        nc.sync.dma_start(out=wb[:, :],
                                          in_=w0_d[ic, 0, half, :, 0:NB])
                        wbs.append(wb)
                    kan_matmuls(pts, wbs, fics[ic], ic, 8, js=(0,))
                # phase B: spline chains + j=1..8 matmuls, streamed weights
                for ic in range(8):
                    fics[ic][1:] = emit_spline(xts[ic][:, :], f"a{ic}")
                    wsr = []
                    for half in range(2):
                        # full-size tile sharing the ws tag family; only the
                        # j>=1 region is loaded (j=0 went via the wb tiles)
                        wt = wpool.tile([128, 9 * NB], F16, tag="ws",
                                        bufs=2, name=f"wsr{ic}_{half}")
                        nc.sync.dma_start(out=wt[:, NB:9 * NB],
                                          in_=w0_d[ic, 0, half, :, NB:9 * NB])
                        wsr.append(wt)
                    kan_matmuls(pts, wsr, fics[ic], ic, 8,
                                js=range(1, 9))
            else:
                for ic in range(8):
                    whs = []
                    for half in range(2):
                        wt = wpool.tile([128, 9 * NB], F16, tag="ws",
                                        bufs=2, name=f"w1g_{ic}_{half}")
                        nc.sync.dma_start(out=wt[:, :],
                                          in_=w0_d[ic, 1, half, :, :])
                        whs.append(wt)
                    kan_matmuls(pts, whs, fics[ic], ic, 8)
                    # recompute feats for layer 1 while og1 matmuls drain
                    fics[ic] = [emit_base(h0ts[ic][:, :], f"b{ic}")] + \
                        emit_spline(h0ts[ic][:, :], f"b{ic}")
            for oc in range(8):
                h0t = fpool.tile([128, NB], F16, tag=f"h0_{og * 8 + oc}",
                                 name=f"h0t{og * 8 + oc}")
                nc.scalar.activation(h0t[:, :], pts[oc][:, :], AF.Copy,
                                     bias=0.0, scale=1.0)
                h0ts[og * 8 + oc] = h0t

        # head weights: small, load while layer 1 runs
        dwt = opool.tile([128, 8, L], F16, name="dwt")
        for ic in range(8):
            nc.sync.dma_start(out=dwt[:, ic, :], in_=dw_d[ic, :, :])
        dbt = opool.tile([128, 4], F32, name="dbt")
        nc.sync.dma_start(out=dbt[:, :], in_=db_d[:, :])

        # ---- Layer 1: 8 out chunks, 16 contraction chunks ----
        pts = [psum.tile([128, NB], F32, tag=f"bank{oc}", name=f"psB{oc}")
               for oc in range(8)]
        for ic in range(16):
            if ic >= 8:
                fics[ic - 8] = [emit_base(h0ts[ic][:, :], f"c{ic - 8}")] + \
                    emit_spline(h0ts[ic][:, :], f"c{ic - 8}")
            whs = []
            for half in range(2):
                wt = wpool.tile([128, 9 * NB], F16, tag="ws", bufs=2,
                                name=f"w1_{ic}_{half}")
                nc.sync.dma_start(out=wt[:, :], in_=w1_d[ic, half, :, :])
                whs.append(wt)
            kan_matmuls(pts, whs, fics[ic % 8], ic, 16)

        # ---- Head: relu(h1) @ dw.T + db ----
        rls = []
        for oc in range(8):
            rl = opool.tile([128, NB], F16, tag=f"rl{oc}", name=f"rl{oc}")
            if oc % 2 == 0:
                nc.scalar.activation(rl[:, :], pts[oc][:, :], AF.Relu,
                                     bias=bias_ap(0.0), scale=1.0)
            else:
                nc.vector.tensor_scalar(rl[:, :], pts[oc][:, :], 0.0, None,
                                        ALU.max)
            rls.append(rl)
        hpts = [psum.tile([128, NB], F32, tag=f"bank{lc}", name=f"psC{lc}")
                for lc in range(4)]
        for ic in range(8):
            for lc in range(4):
                nc.tensor.matmul(hpts[lc][:, :],
                                 dwt[:, ic, lc * 128:(lc + 1) * 128],
                                 rls[ic][:, :],
                                 start=(ic == 0), stop=(ic == 7))
        for lc in range(4):
            ot = opool.tile([128, NB], F32, tag="ot", bufs=2, name=f"ot{lc}")
            nc.scalar.activation(ot[:, :], hpts[lc][:, :], AF.Identity,
                                 bias=dbt[:, lc:lc + 1], scale=1.0)
            nc.sync.dma_start(out=o_d[lc, :, :], in_=ot[:, :])

    nc.compile()
    return nc


def _prep_weights(bw0, sw0, ss0, bw1, sw1, ss1, dw, db):
    # KAN layer weights: slot j=0 -> base weight, j=1+k -> sw*ss/6
    W0 = np.empty((D_IN, 9, H0), np.float32)
    W0[:, 0, :] = bw0.T
    W0[:, 1:, :] = (sw0 * (ss0[:, :, None] / 6.0)).transpose(1, 2, 0)
    w0 = np.ascontiguousarray(
        W0.reshape(8, 128, 9, 2, 2, 4, 128)
          .transpose(0, 3, 4, 1, 2, 5, 6)
          .reshape(8, 2, 2, 128, 9 * NB)).astype(np.float16)
    W1 = np.empty((H0, 9, H1), np.float32)
    W1[:, 0, :] = bw1.T
    W1[:, 1:, :] = (sw1 * (ss1[:, :, None] / 6.0)).transpose(1, 2, 0)
    w1 = np.ascontiguousarray(
        W1.reshape(16, 128, 9, 2, 4, 128)
          .transpose(0, 3, 1, 2, 4, 5)
          .reshape(16, 2, 128, 9 * NB)).astype(np.float16)
    dwt = np.ascontiguousarray(dw.T.reshape(8, 128, L)).astype(np.float16)
    dbt = np.ascontiguousarray(db.reshape(4, 128).T.astype(np.float32))
    return w0, w1, dwt, dbt


def kernel(x, bw0, sw0, ss0, bw1, sw1, ss1, dw, db):
    if "nc" not in _cache:
        _cache["nc"] = _build_full()
    nc = _cache["nc"]
    w0, w1, dwt, dbt = _prep_weights(
        np.asarray(bw0, np.float32), np.asarray(sw0, np.float32),
        np.asarray(ss0, np.float32), np.asarray(bw1, np.float32),
        np.asarray(sw1, np.float32), np.asarray(ss1, np.float32),
        np.asarray(dw, np.float32), np.asarray(db, np.float32))
    xT = np.ascontiguousarray(np.asarray(x, np.float32).T).astype(np.float16)
    in_maps = []
    for c in range(NCORES):
        xc = np.ascontiguousarray(
            xT[:, c * BC:(c + 1) * BC].reshape(8, 128, BC))
        in_maps.append({"x_d": xc, "w0_d": w0, "w1_d": w1,
                        "dw_d": dwt, "db_d": dbt})
    _cache["in_maps"] = in_maps
    res = run_bass_kernel_spmd(nc, in_maps, list(range(NCORES)))
    out = np.empty((B, L), np.float32)
    for c in range(NCORES):
        oc = res.results[c]["o_d"]          # (4, 128, BC)
        out[c * BC:(c + 1) * BC, :] = oc.reshape(L, BC).T
    return out


# revision 19
# speedup vs baseline: 1.3905x; 1.3499x over previous
"""KAN encoder (2 KAN layers + relu + linear head) on 8 trn2 NeuronCores.

Strategy: data-parallel on batch (512 rows/core), all weights replicated,
feature-on-partition / batch-on-free layout throughout (no transposes).

The spline path is a dense matmul over (in*9) with B-spline bases computed
via the exact identity

    bases_k(x) * 6 = a^3 - 4*e^3,   a = relu(2 - s_k),  e = relu(a - 1),
    s_k = |2.5*x + 3.5 - k|

(the 1/6 and the spline_scaler are folded into the weights host-side; the
*4 is realized as (2e)^2 * e).

Performance notes vs the f32 baseline (720us):
  * weights, x and all feature tensors are fp16: halves HBM traffic and
    matmuls still run at 1 cycle/row (same as f32r).
  * the basis slices share relu scale/bias, so the relu/cube chain runs as
    WIDE [128, 4*512] ops: ACT does the 8 per-k Abs + silu + packed relus,
    DVE runs packed fp16 tensor_scalar (4x_2p) / tensor_tensor (2x_1p) ops.
    The f32 baseline was vector-bound (DVE 93% busy at 533ns per
    tensor_tensor); here DVE sits at ~60% and the PE at ~93%.
  * layer-0 features are computed once and kept in SBUF for both output
    halves (the baseline recomputed them).
  * layer-0 og0 runs in two phases: all j=0 (silu-path) matmuls for the 8
    chunks first - they only need silu(x) and a small weight block - so
    the PE starts within ~4us while the spline chains fill the pipeline.
  * features live in 3 tiles per chunk (silu slot + two 4-slot spline
    halves) and matmuls run j-major, so matmuls start as soon as each
    piece is ready instead of waiting for the full feature tile.
"""
import numpy as np
from contextlib import ExitStack

from concourse import bacc, tile, mybir
from concourse.bass_utils import run_bass_kernel_spmd

F32 = mybir.dt.float32
F16 = mybir.dt.float16
AF = mybir.ActivationFunctionType
ALU = mybir.AluOpType

B, D_IN, H0, H1, L = 4096, 1024, 2048, 1024, 512
NCORES = 8
BC = B // NCORES          # 512 batch cols per core
NB = 512                  # free-dim (batch) tile = full per-core batch
HW = 4 * NB               # half of the 8 spline slots

_cache = {}


def _build_full():
    nc = bacc.Bacc("TRN2", target_bir_lowering=False, debug=False,
                   num_devices=NCORES)

    x_d = nc.dram_tensor("x_d", [8, 128, BC], F16, kind="ExternalInput")
    w0_d = nc.dram_tensor("w0_d", [8, 2, 2, 128, 9 * NB], F16,
                          kind="ExternalInput")
    w1_d = nc.dram_tensor("w1_d", [16, 2, 128, 9 * NB], F16,
                          kind="ExternalInput")
    dw_d = nc.dram_tensor("dw_d", [8, 128, L], F16, kind="ExternalInput")
    db_d = nc.dram_tensor("db_d", [128, 4], F32, kind="ExternalInput")
    o_d = nc.dram_tensor("o_d", [4, 128, BC], F32, kind="ExternalOutput")

    with tile.TileContext(nc) as tc, ExitStack() as ctx:
        psum = ctx.enter_context(tc.tile_pool(name="psum", bufs=1,
                                              space="PSUM"))
        fpool = ctx.enter_context(tc.tile_pool(name="fpool", bufs=1))
        wpool = ctx.enter_context(tc.tile_pool(name="wpool", bufs=1))
        ipool = ctx.enter_context(tc.tile_pool(name="ipool", bufs=1))
        opool = ctx.enter_context(tc.tile_pool(name="opool", bufs=1))

        fics = [None] * 8     # current feats tiles (base, sp0, sp1)
        h0ts = [None] * 16    # layer-0 output chunks

        bias_tiles = {}

        def bias_ap(val):
            val = float(val)
            if val not in bias_tiles:
                t = opool.tile([128, 1], F32, tag=f"b{len(bias_tiles)}",
                               name=f"bias{len(bias_tiles)}")
                nc.gpsimd.memset(t[:, :], val)
                bias_tiles[val] = t
            return bias_tiles[val][:, :]

        def emit_base(src_ap, tagp):
            """base slot (128, NB) = silu(src)."""
            fb = fpool.tile([128, NB], F16, tag=f"fb{tagp[-1]}",
                            name=f"fb{tagp}")
            nc.scalar.activation(fb[:, :], src_ap, AF.Silu,
                                 bias=bias_ap(0.0), scale=1.0)
            return fb

        def emit_spline(src_ap, tagp):
            """Returns [sp0, sp1]: spP (128,4*NB) slot q -> 6*bases_{4P+q}."""
            out = []
            for p in range(2):
                # sp is ACT-produced and ACT-consumed: bufs=1 cannot stall
                # the cross-engine pipeline; av is read by DVE so it gets 2.
                sp = ipool.tile([128, HW], F16, tag=f"sp{p}", bufs=1,
                                name=f"sp{p}_{tagp}")
                for q in range(4):
                    k = 4 * p + q
                    nc.scalar.activation(sp[:, q * NB:(q + 1) * NB], src_ap,
                                         AF.Abs, bias=bias_ap(3.5 - k),
                                         scale=2.5)
                av = ipool.tile([128, HW], F16, tag=f"av{p}", bufs=2,
                                name=f"av{p}_{tagp}")
                nc.scalar.activation(av[:, :], sp[:, :], AF.Relu,
                                     bias=bias_ap(2.0), scale=-1.0)
                et = ipool.tile([128, HW], F16, tag=f"et{p}", bufs=1,
                                name=f"et{p}_{tagp}")
                nc.vector.tensor_scalar(et[:, :], av[:, :], 1.0, 0.0,
                                        ALU.subtract, ALU.max)
                a2 = ipool.tile([128, HW], F16, tag=f"p{p}", bufs=1,
                                name=f"a2{p}_{tagp}")
                nc.vector.tensor_tensor(a2[:, :], av[:, :], av[:, :],
                                        ALU.mult)
                a3 = ipool.tile([128, HW], F16, tag=f"q{p}", bufs=1,
                                name=f"a3{p}_{tagp}")
                nc.vector.tensor_tensor(a3[:, :], a2[:, :], av[:, :],
                                        ALU.mult)
                gt = ipool.tile([128, HW], F16, tag=f"p{p}", bufs=1,
                                name=f"gt{p}_{tagp}")
                nc.vector.tensor_scalar(gt[:, :], et[:, :], 2.0, None,
                                        ALU.mult)
                e2 = ipool.tile([128, HW], F16, tag=f"r{p}", bufs=1,
                                name=f"e2{p}_{tagp}")
                nc.vector.tensor_tensor(e2[:, :], gt[:, :], gt[:, :],
                                        ALU.mult)
                e3 = ipool.tile([128, HW], F16, tag=f"p{p}", bufs=1,
                                name=f"e3{p}_{tagp}")
                nc.vector.tensor_tensor(e3[:, :], e2[:, :], et[:, :],
                                        ALU.mult)
                fs = fpool.tile([128, HW], F16, tag=f"fs{p}_{tagp[-1]}",
                                name=f"fs{p}_{tagp}")
                nc.vector.tensor_tensor(fs[:, :], a3[:, :], e3[:, :],
                                        ALU.subtract)
                out.append(fs)
            return out

        def kan_matmuls(pts, whs, fic, ic, n_ic, js=range(9), j_off=0,
                        has_start=True):
            # j-major so early-ready features feed the PE first
            for j in js:
                if j == 0:
                    rhs = fic[0][:, :]
                else:
                    rhs = fic[1 + (j - 1) // 4][
                        :, ((j - 1) % 4) * NB:((j - 1) % 4 + 1) * NB]
                for half in range(2):
                    ws = whs[half]
                    for oc4 in range(4):
                        oc = half * 4 + oc4
                        base = (j - j_off) * NB + oc4 * 128
                        nc.tensor.matmul(
                            pts[oc][:, :], ws[:, base:base + 128], rhs,
                            start=(has_start and ic == 0 and j == 0),
                            stop=(ic == n_ic - 1 and j == 8))

        # ---- Layer 0: out split in two groups of 8 PSUM banks ----
        for og in range(2):
            pts = [psum.tile([128, NB], F32, tag=f"bank{oc}",
                             name=f"psA{og}_{oc}") for oc in range(8)]
            if og == 0:
                # phase A: j=0 (silu-path) matmuls only; tiny weight DMAs
                for ic in range(8):
                    xt = ipool.tile([128, NB], F16, tag="xt", bufs=2,
                                    name=f"xtA{ic}")
                    nc.sync.dma_start(out=xt[:, :], in_=x_d[ic, :, :])
                    fics[ic] = [emit_base(xt[:, :], f"a{ic}"), None, None]
                    wbs = []
                    for half in range(2):
                        wb = wpool.tile([128, NB], F16, tag="wb", bufs=4,
                                        name=f"wb{ic}_{half}")
                        nc.sync.dma_start(out=wb[:, :],
                                          in_=w0_d[ic, 0, half, :, 0:NB])
                        wbs.append(wb)
                    kan_matmuls(pts, wbs, fics[ic], ic, 8, js=(0,))
                # phase B: spline chains + j=1..8 matmuls, streamed weights
                for ic in range(8):
                    xt = ipool.tile([128, NB], F16, tag="xt", bufs=2,
                                    name=f"xtB{ic}")
                    nc.sync.dma_start(out=xt[:, :], in_=x_d[ic, :, :])
                    fics[ic][1:] = emit_spline(xt[:, :], f"a{ic}")
                    wsr = []
                    for half in range(2):
                        # full-size tile sharing the ws tag family; only the
                        # j>=1 region is loaded (j=0 went via the wb tiles)
                        wt = wpool.tile([128, 9 * NB], F16, tag="ws",
                                        bufs=4, name=f"wsr{ic}_{half}")
                        nc.sync.dma_start(out=wt[:, NB:9 * NB],
                                          in_=w0_d[ic, 0, half, :, NB:9 * NB])
                        wsr.append(wt)
                    kan_matmuls(pts, wsr, fics[ic], ic, 8,
                                js=range(1, 9))
            else:
                for ic in range(8):
                    whs = []
                    for half in range(2):
                        wt = wpool.tile([128, 9 * NB], F16, tag="ws",
                                        bufs=4, name=f"w1g_{ic}_{half}")
                        nc.sync.dma_start(out=wt[:, :],
                                          in_=w0_d[ic, 1, half, :, :])
                        whs.append(wt)
                    kan_matmuls(pts, whs, fics[ic], ic, 8)
                    # recompute feats for layer 1 while og1 matmuls drain
                    fics[ic] = [emit_base(h0ts[ic][:, :], f"b{ic}")] + \
                        emit_spline(h0ts[ic][:, :], f"b{ic}")
            for oc in range(8):
                h0t = fpool.tile([128, NB], F16, tag=f"h0_{og * 8 + oc}",
                                 name=f"h0t{og * 8 + oc}")
                nc.scalar.activation(h0t[:, :], pts[oc][:, :], AF.Copy,
                                     bias=0.0, scale=1.0)
                h0ts[og * 8 + oc] = h0t

        # head weights: small, load while layer 1 runs
        dwt = opool.tile([128, 8, L], F16, name="dwt")
        for ic in range(8):
            nc.sync.dma_start(out=dwt[:, ic, :], in_=dw_d[ic, :, :])
        dbt = opool.tile([128, 4], F32, name="dbt")
        nc.sync.dma_start(out=dbt[:, :], in_=db_d[:, :])

        # ---- Layer 1: 8 out chunks, 16 contraction chunks ----
        pts = [psum.tile([128, NB], F32, tag=f"bank{oc}", name=f"psB{oc}")
               for oc in range(8)]
        for ic in range(16):
            if ic >= 8:
                fics[ic - 8] = [emit_base(h0ts[ic][:, :], f"c{ic - 8}")] + \
                    emit_spline(h0ts[ic][:, :], f"c{ic - 8}")
            whs = []
            for half in range(2):
                wt = wpool.tile([128, 9 * NB], F16, tag="ws", bufs=4,
                                name=f"w1_{ic}_{half}")
                nc.sync.dma_start(out=wt[:, :], in_=w1_d[ic, half, :, :])
                whs.append(wt)
            kan_matmuls(pts, whs, fics[ic % 8], ic, 16)

        # ---- Head: relu(h1) @ dw.T + db ----
        rls = []
        for oc in range(8):
            rl = opool.tile([128, NB], F16, tag=f"rl{oc}", name=f"rl{oc}")
            if oc % 2 == 0:
                nc.scalar.activation(rl[:, :], pts[oc][:, :], AF.Relu,
                                     bias=bias_ap(0.0), scale=1.0)
            else:
                nc.vector.tensor_scalar(rl[:, :], pts[oc][:, :], 0.0, None,
                                        ALU.max)
            rls.append(rl)
        hpts = [psum.tile([128, NB], F32, tag=f"bank{lc}", name=f"psC{lc}")
                for lc in range(4)]
        for ic in range(8):
            for lc in range(4):
                nc.tensor.matmul(hpts[lc][:, :],
                                 dwt[:, ic, lc * 128:(lc + 1) * 128],
                                 rls[ic][:, :],
                                 start=(ic == 0), stop=(ic == 7))
        for lc in range(4):
            ot = opool.tile([128, NB], F32, tag="ot", bufs=2, name=f"ot{lc}")
            nc.scalar.activation(ot[:, :], hpts[lc][:, :], AF.Identity,
                                 bias=dbt[:, lc:lc + 1], scale=1.0)
            nc.sync.dma_start(out=o_d[lc, :, :], in_=ot[:, :])

    nc.compile()
    return nc


def _prep_weights(bw0, sw0, ss0, bw1, sw1, ss1, dw, db):
    # KAN layer weights: slot j=0 -> base weight, j=1+k -> sw*ss/6
    W0 = np.empty((D_IN, 9, H0), np.float32)
    W0[:, 0, :] = bw0.T
    W0[:, 1:, :] = (sw0 * (ss0[:, :, None] / 6.0)).transpose(1, 2, 0)
    w0 = np.ascontiguousarray(
        W0.reshape(8, 128, 9, 2, 2, 4, 128)
          .transpose(0, 3, 4, 1, 2, 5, 6)
          .reshape(8, 2, 2, 128, 9 * NB)).astype(np.float16)
    W1 = np.empty((H0, 9, H1), np.float32)
    W1[:, 0, :] = bw1.T
    W1[:, 1:, :] = (sw1 * (ss1[:, :, None] / 6.0)).transpose(1, 2, 0)
    w1 = np.ascontiguousarray(
        W1.reshape(16, 128, 9, 2, 4, 128)
          .transpose(0, 3, 1, 2, 4, 5)
          .reshape(16, 2, 128, 9 * NB)).astype(np.float16)
    dwt = np.ascontiguousarray(dw.T.reshape(8, 128, L)).astype(np.float16)
    dbt = np.ascontiguousarray(db.reshape(4, 128).T.astype(np.float32))
    return w0, w1, dwt, dbt


def kernel(x, bw0, sw0, ss0, bw1, sw1, ss1, dw, db):
    if "nc" not in _cache:
        _cache["nc"] = _build_full()
    nc = _cache["nc"]
    w0, w1, dwt, dbt = _prep_weights(
        np.asarray(bw0, np.float32), np.asarray(sw0, np.float32),
        np.asarray(ss0, np.float32), np.asarray(bw1, np.float32),
        np.asarray(sw1, np.float32), np.asarray(ss1, np.float32),
        np.asarray(dw, np.float32), np.asarray(db, np.float32))
    xT = np.ascontiguousarray(np.asarray(x, np.float32).T).astype(np.float16)
    in_maps = []
    for c in range(NCORES):
        xc = np.ascontiguousarray(
            xT[:, c * BC:(c + 1) * BC].reshape(8, 128, BC))
        in_maps.append({"x_d": xc, "w0_d": w0, "w1_d": w1,
                        "dw_d": dwt, "db_d": dbt})
    _cache["in_maps"] = in_maps
    res = run_bass_kernel_spmd(nc, in_maps, list(range(NCORES)))
    out = np.empty((B, L), np.float32)
    for c in range(NCORES):
        oc = res.results[c]["o_d"]          # (4, 128, BC)
        out[c * BC:(c + 1) * BC, :] = oc.reshape(L, BC).T
    return out


# revision 20
# speedup vs baseline: 1.4190x; 1.0205x over previous
"""KAN encoder (2 KAN layers + relu + linear head) on 8 trn2 NeuronCores.

Strategy: data-parallel on batch (512 rows/core), all weights replicated,
feature-on-partition / batch-on-free layout throughout (no transposes).

The spline path is a dense matmul over (in*9) with B-spline bases computed
via the exact identity

    bases_k(x) * 6 = a^3 - 4*e^3,   a = relu(2 - s_k),  e = relu(a - 1),
    s_k = |2.5*x + 3.5 - k|

(the 1/6 and the spline_scaler are folded into the weights host-side; the
*4 is realized as (2e)^2 * e).

Performance notes vs the f32 baseline (720us -> 544us):
  * weights, x and all feature tensors are fp16: halves HBM traffic and
    matmuls still run at 1 cycle/row (same as f32r).
  * the basis slices share relu scale/bias, so the relu/cube chain runs as
    WIDE [128, 8*512] ops: ACT does the 8 per-k Abs + silu + packed relu,
    DVE runs packed fp16 tensor_scalar (4x_2p) / tensor_tensor (2x_1p)
    ops.  The f32 baseline was vector-bound (DVE 93% busy at 533ns per
    f32 tensor_tensor); here DVE sits at ~60% and the PE at ~93%.
  * e = relu(a-1) removes the second packed ACT relu; the *4 on e^3 is
    realized by squaring 2e, keeping everything in cheap packed DVE ops.
  * layer-0 features are computed once and kept in SBUF for both output
    halves (the baseline recomputed them); layer-1 features for the first
    8 chunks are computed during og1's matmuls so layer 1 starts hot.
"""
import numpy as np
from contextlib import ExitStack

from concourse import bacc, tile, mybir
from concourse.bass_utils import run_bass_kernel_spmd

F32 = mybir.dt.float32
F16 = mybir.dt.float16
AF = mybir.ActivationFunctionType
ALU = mybir.AluOpType

B, D_IN, H0, H1, L = 4096, 1024, 2048, 1024, 512
NCORES = 8
BC = B // NCORES          # 512 batch cols per core
NB = 512                  # free-dim (batch) tile = full per-core batch
CBRT4 = float(4.0 ** (1.0 / 3.0))

_cache = {}


def _build_full():
    nc = bacc.Bacc("TRN2", target_bir_lowering=False, debug=False,
                   num_devices=NCORES)

    x_d = nc.dram_tensor("x_d", [8, 128, BC], F16, kind="ExternalInput")
    w0_d = nc.dram_tensor("w0_d", [8, 2, 2, 128, 9 * NB], F16,
                          kind="ExternalInput")
    w1_d = nc.dram_tensor("w1_d", [16, 2, 128, 9 * NB], F16,
                          kind="ExternalInput")
    dw_d = nc.dram_tensor("dw_d", [8, 128, L], F16, kind="ExternalInput")
    db_d = nc.dram_tensor("db_d", [128, 4], F32, kind="ExternalInput")
    o_d = nc.dram_tensor("o_d", [4, 128, BC], F32, kind="ExternalOutput")

    with tile.TileContext(nc) as tc, ExitStack() as ctx:
        psum = ctx.enter_context(tc.tile_pool(name="psum", bufs=1,
                                              space="PSUM"))
        fpool = ctx.enter_context(tc.tile_pool(name="fpool", bufs=1))
        wpool = ctx.enter_context(tc.tile_pool(name="wpool", bufs=1))
        ipool = ctx.enter_context(tc.tile_pool(name="ipool", bufs=1))
        opool = ctx.enter_context(tc.tile_pool(name="opool", bufs=1))

        fics = [None] * 8     # current feats tile per contraction chunk
        h0ts = [None] * 16    # layer-0 output chunks

        bias_tiles = {}

        def bias_ap(val):
            val = float(val)
            if val not in bias_tiles:
                t = opool.tile([128, 1], F32, tag=f"b{len(bias_tiles)}",
                               name=f"bias{len(bias_tiles)}")
                nc.gpsimd.memset(t[:, :], val)
                bias_tiles[val] = t
            return bias_tiles[val][:, :]

        def emit_feats(src_ap, fic, tagp):
            """fic (128, 9*NB) f16: j=0 silu(src); j=1+k -> 6*bases_k(src)."""
            sp = ipool.tile([128, 8 * NB], F16, tag="sp", bufs=2,
                            name=f"sp{tagp}")
            av = ipool.tile([128, 8 * NB], F16, tag="av", bufs=2,
                            name=f"av{tagp}")
            et = ipool.tile([128, 8 * NB], F16, tag="et", bufs=1,
                            name=f"et{tagp}")
            nc.scalar.activation(fic[:, 0:NB], src_ap, AF.Silu,
                                 bias=bias_ap(0.0), scale=1.0)
            for k in range(8):
                nc.scalar.activation(sp[:, k * NB:(k + 1) * NB], src_ap,
                                     AF.Abs, bias=bias_ap(3.5 - k), scale=2.5)
            nc.scalar.activation(av[:, :], sp[:, :], AF.Relu,
                                 bias=bias_ap(2.0), scale=-1.0)
            nc.vector.tensor_scalar(et[:, :], av[:, :], 1.0, 0.0,
                                    ALU.subtract, ALU.max)
            a2 = ipool.tile([128, 8 * NB], F16, tag="p", bufs=1,
                            name=f"a2{tagp}")
            a3 = ipool.tile([128, 8 * NB], F16, tag="q", bufs=1,
                            name=f"a3{tagp}")
            nc.vector.tensor_tensor(a2[:, :], av[:, :], av[:, :], ALU.mult)
            nc.vector.tensor_tensor(a3[:, :], a2[:, :], av[:, :], ALU.mult)
            gt = ipool.tile([128, 8 * NB], F16, tag="p", bufs=1,
                            name=f"gt{tagp}")
            nc.vector.tensor_scalar(gt[:, :], et[:, :], 2.0, None, ALU.mult)
            e2 = ipool.tile([128, 8 * NB], F16, tag="r", bufs=1,
                            name=f"e2{tagp}")
            nc.vector.tensor_tensor(e2[:, :], gt[:, :], gt[:, :], ALU.mult)
            e3 = ipool.tile([128, 8 * NB], F16, tag="p", bufs=1,
                            name=f"e3{tagp}")
            nc.vector.tensor_tensor(e3[:, :], e2[:, :], et[:, :], ALU.mult)
            nc.vector.tensor_tensor(fic[:, NB:9 * NB], a3[:, :], e3[:, :],
                                    ALU.subtract)

        def kan_matmuls(pts, wsl, fic, ic, n_ic):
            for half in range(2):
                for oc4 in range(4):
                    oc = half * 4 + oc4
                    base = oc4 * 128
                    for j in range(9):
                        nc.tensor.matmul(
                            pts[oc][:, :],
                            wsl[half][:, j * NB + base:j * NB + base + 128],
                            fic[:, j * NB:(j + 1) * NB],
                            start=(ic == 0 and j == 0),
                            stop=(ic == n_ic - 1 and j == 8))

        # ---- Layer 0: out split in two groups of 8 PSUM banks ----
        for og in range(2):
            pts = [psum.tile([128, NB], F32, tag=f"bank{oc}",
                             name=f"psA{og}_{oc}") for oc in range(8)]
            for ic in range(8):
                if og == 0:
                    xt = ipool.tile([128, NB], F16, tag="xt", bufs=2,
                                    name=f"xt{ic}")
                    nc.sync.dma_start(out=xt[:, :], in_=x_d[ic, :, :])
                    fic = fpool.tile([128, 9 * NB], F16, tag=f"fic{ic}",
                                     name=f"f0_{ic}")
                    emit_feats(xt[:, :], fic, f"0_{ic}")
                    fics[ic] = fic
                wsl = []
                for half in range(2):
                    wt = wpool.tile([128, 9 * NB], F16, tag="w", bufs=3,
                                    name=f"w0_{og}_{ic}_{half}")
                    nc.sync.dma_start(out=wt[:, :], in_=w0_d[ic, og, half])
                    wsl.append(wt)
                kan_matmuls(pts, wsl, fics[ic], ic, 8)
                if og == 1:
                    # recompute feats for layer 1 while og1 matmuls drain
                    fic = fpool.tile([128, 9 * NB], F16, tag=f"fic{ic}",
                                     name=f"f1_{ic}")
                    emit_feats(h0ts[ic][:, :], fic, f"1_{ic}")
                    fics[ic] = fic
            for oc in range(8):
                h0t = fpool.tile([128, NB], F16, tag=f"h0_{og * 8 + oc}",
                                 name=f"h0t{og * 8 + oc}")
                nc.scalar.activation(h0t[:, :], pts[oc][:, :], AF.Copy,
                                     bias=0.0, scale=1.0)
                h0ts[og * 8 + oc] = h0t

        # head weights: small, load while layer 1 runs
        dwt = opool.tile([128, 8, L], F16, name="dwt")
        for ic in range(8):
            nc.sync.dma_start(out=dwt[:, ic, :], in_=dw_d[ic, :, :])
        dbt = opool.tile([128, 4], F32, name="dbt")
        nc.sync.dma_start(out=dbt[:, :], in_=db_d[:, :])

        # ---- Layer 1: 8 out chunks, 16 contraction chunks ----
        pts = [psum.tile([128, NB], F32, tag=f"bank{oc}", name=f"psB{oc}")
               for oc in range(8)]
        for ic in range(16):
            if ic >= 8:
                fic = fpool.tile([128, 9 * NB], F16, tag=f"fic{ic - 8}",
                                 name=f"f1_{ic}")
                emit_feats(h0ts[ic][:, :], fic, f"1_{ic}")
                fics[ic - 8] = fic
            wsl = []
            for half in range(2):
                wt = wpool.tile([128, 9 * NB], F16, tag="w", bufs=3,
                                name=f"w1_{ic}_{half}")
                nc.sync.dma_start(out=wt[:, :], in_=w1_d[ic, half])
                wsl.append(wt)
            kan_matmuls(pts, wsl, fics[ic % 8], ic, 16)

        # ---- Head: relu(h1) @ dw.T + db ----
        rl = opool.tile([128, 8 * NB], F16, name="rl")
        for oc in range(8):
            nc.scalar.activation(rl[:, oc * NB:(oc + 1) * NB],
                                 pts[oc][:, :], AF.Relu,
                                 bias=bias_ap(0.0), scale=1.0)
        for lc in range(4):
            pt = psum.tile([128, NB], F32, tag=f"bank{lc}", name=f"psC{lc}")
            for ic in range(8):
                nc.tensor.matmul(pt[:, :], dwt[:, ic, lc * 128:(lc + 1) * 128],
                                 rl[:, ic * NB:(ic + 1) * NB],
                                 start=(ic == 0), stop=(ic == 7))
            ot = opool.tile([128, NB], F32, tag="ot", bufs=2, name=f"ot{lc}")
            nc.scalar.activation(ot[:, :], pt[:, :], AF.Identity,
                                 bias=dbt[:, lc:lc + 1], scale=1.0)
            nc.sync.dma_start(out=o_d[lc, :, :], in_=ot[:, :])

    nc.compile()
    return nc


def _prep_weights(bw0, sw0, ss0, bw1, sw1, ss1, dw, db):
    # KAN layer weights: slot j=0 -> base weight, j=1+k -> sw*ss/6
    W0 = np.empty((D_IN, 9, H0), np.float32)
    W0[:, 0, :] = bw0.T
    W0[:, 1:, :] = (sw0 * (ss0[:, :, None] / 6.0)).transpose(1, 2, 0)
    w0 = np.ascontiguousarray(
        W0.reshape(8, 128, 9, 2, 2, 4, 128)
          .transpose(0, 3, 4, 1, 2, 5, 6)
          .reshape(8, 2, 2, 128, 9 * NB)).astype(np.float16)
    W1 = np.empty((H0, 9, H1), np.float32)
    W1[:, 0, :] = bw1.T
    W1[:, 1:, :] = (sw1 * (ss1[:, :, None] / 6.0)).transpose(1, 2, 0)
    w1 = np.ascontiguousarray(
        W1.reshape(16, 128, 9, 2, 4, 128)
          .transpose(0, 3, 1, 2, 4, 5)
          .reshape(16, 2, 128, 9 * NB)).astype(np.float16)
    dwt = np.ascontiguousarray(dw.T.reshape(8, 128, L)).astype(np.float16)
    dbt = np.ascontiguousarray(db.reshape(4, 128).T.astype(np.float32))
    return w0, w1, dwt, dbt


def kernel(x, bw0, sw0, ss0, bw1, sw1, ss1, dw, db):
    if "nc" not in _cache:
        _cache["nc"] = _build_full()
    nc = _cache["nc"]
    w0, w1, dwt, dbt = _prep_weights(
        np.asarray(bw0, np.float32), np.asarray(sw0, np.float32),
        np.asarray(ss0, np.float32), np.asarray(bw1, np.float32),
        np.asarray(sw1, np.float32), np.asarray(ss1, np.float32),
        np.asarray(dw, np.float32), np.asarray(db, np.float32))
    xT = np.ascontiguousarray(np.asarray(x, np.float32).T).astype(np.float16)
    in_maps = []
    for c in range(NCORES):
        xc = np.ascontiguousarray(
            xT[:, c * BC:(c + 1) * BC].reshape(8, 128, BC))
        in_maps.append({"x_d": xc, "w0_d": w0, "w1_d": w1,
                        "dw_d": dwt, "db_d": dbt})
    _cache["in_maps"] = in_maps
    res = run_bass_kernel_spmd(nc, in_maps, list(range(NCORES)))
    out = np.empty((B, L), np.float32)
    for c in range(NCORES):
        oc = res.results[c]["o_d"]          # (4, 128, BC)
        out[c * BC:(c + 1) * BC, :] = oc.reshape(L, BC).T
    return out
